# revision 2
# baseline (speedup 1.0000x reference)
"""Trainium2 Bass kernel for the GRU encoder problem (nn_Encoder).

Computation: x = embedding[source]; gi = x @ w_ih.T + b_ih; then a GRU
recurrence over T=128 steps producing enc_outputs [T, B, H].

Strategy: data-parallel over batch across 8 NeuronCores (B=64 -> 8 rows/core,
embedding + GRU weights replicated). Inside each core everything runs in a
"transposed" layout with gate/hidden dims on SBUF partitions and batch on the
free dim, so the sequential recurrence's per-step matmuls put gates on PSUM
partitions and the elementwise gate math uses all 128 lanes:

  phase A: dma_gather   x[tok, :] = embedding[source[tok]]   (tok t-major)
  phase B: PE-transpose x -> xT, rounded to float32r
  phase C: load w_ih.T, cast float32r
  phase D: gi GEMM (float32r, full PE rate): giT = w_ih.T^T @ xT + biases
           -> DRAM [128, 30, NTOK]
  phase E: recurrence, per step t:
             G[p, j*8+b] = sum_k whhT[:,k,jblk]^T @ h[:, k*8+b]   (bf16)
             rz = sigmoid(G_rz + gi_rz)
             n  = tanh((G_n + b_hh_n) * r + gi_n)
             h' = (h - n) * z + n         -> outT[t]
The h' tile in layout [128, k*8+b] is exactly the next step's matmul rhs, so
no per-step transposes are needed.
"""
import numpy as np

V, E, H, B, T = 32000, 1024, 1280, 64, 128
BL = 8            # batch rows per core
G3 = 3 * H        # 3840
NJ = G3 // 128    # 30 gate blocks
NK = H // 128     # 10 hidden blocks
NE = E // 128     # 8 embedding blocks
N_CORES = 8

_CACHE = {}


def _build(nc):
    import concourse.mybir as mybir
    import concourse.tile as tile

    F32 = mybir.dt.float32
    F32R = mybir.dt.float32r
    BF16 = mybir.dt.bfloat16
    I16 = mybir.dt.int16
    AF = mybir.ActivationFunctionType
    OP = mybir.AluOpType

    NTOK = T * BL
    NTC = NTOK // 128

    emb_d = nc.dram_tensor("emb", [V, E], F32, kind="ExternalInput")
    idx_d = nc.dram_tensor("idx", [128, NTOK // 16], I16, kind="ExternalInput")
    wihT_d = nc.dram_tensor("wihT", [E, G3], F32, kind="ExternalInput")
    whhT_d = nc.dram_tensor("whhT", [H, G3], F32, kind="ExternalInput")
    bgi_d = nc.dram_tensor("bgi", [128, NJ], F32, kind="ExternalInput")
    bhhn_d = nc.dram_tensor("bhhn", [128, BL * NK], F32, kind="ExternalInput")
    ident_d = nc.dram_tensor("ident", [128, 128], F32, kind="ExternalInput")
    giT_d = nc.dram_tensor("giT", [128, NJ, NTOK], F32, kind="Internal")
    outT_d = nc.dram_tensor("outT", [T, 128, BL * NK], F32, kind="ExternalOutput")

    with tile.TileContext(nc) as tc:
        with tc.tile_pool(name="const", bufs=1) as cpool:
            bgi = cpool.tile([128, NJ], F32, tag="bgi")
            nc.sync.dma_start(bgi[:, :], bgi_d.ap())
            bhhn = cpool.tile([128, BL * NK], F32, tag="bhhn")
            nc.sync.dma_start(bhhn[:, :], bhhn_d.ap())

            # ---------- phases A-D: gather, transpose, gi GEMM ----------
            with tc.tile_pool(name="gemm", bufs=1) as gp:
                ident = gp.tile([128, 128], F32, tag="ident")
                nc.sync.dma_start(ident[:, :], ident_d.ap())
                idx_sb = gp.tile([128, NTOK // 16], I16, tag="idx")
                nc.sync.dma_start(idx_sb[:, :], idx_d.ap())
                xT = gp.tile([128, NE, NTOK], F32R, tag="xT")
                with tc.tile_pool(name="xp", bufs=1) as xp:
                    x_sb = xp.tile([128, NTC, E], F32, tag="x")
                    nc.gpsimd.dma_gather(
                        x_sb[:, :, :], emb_d.ap(), idx_sb[:, :],
                        num_idxs=NTOK, num_idxs_reg=NTOK, elem_size=E)
                    with tc.tile_pool(name="trps", bufs=4, space="PSUM") as tpp:
                        for c in range(NTC):
                            for e in range(NE):
                                tps = tpp.tile([128, 128], F32, tag="tps")
                                nc.tensor.transpose(
                                    tps[:, :], x_sb[:, c, 128 * e:128 * (e + 1)],
                                    ident[:, :])
                                nc.vector.tensor_copy(
                                    xT[:, e, 128 * c:128 * (c + 1)], tps[:, :])
                wp_ctx = tc.tile_pool(name="wp", bufs=1)
                wp = wp_ctx.__enter__()
                wih = wp.tile([128, NE, G3], F32R, tag="wih")
                for e in range(NE):
                    wsc = wp.tile([128, G3], F32, tag="wsc", bufs=2)
                    nc.sync.dma_start(
                        wsc[:, :], wihT_d.ap()[128 * e:128 * (e + 1), :])
                    nc.vector.tensor_copy(wih[:, e, :], wsc[:, :])
                NN = NTOK // 512
                NW = 512
                with tc.tile_pool(name="gips", bufs=4, space="PSUM") as gpp:
                    for j in range(NJ):
                        for n in range(NN):
                            gps = gpp.tile([128, NW], F32, tag="gps")
                            for e in range(NE):
                                nc.tensor.matmul(
                                    gps[:, :],
                                    wih[:, e, 128 * j:128 * (j + 1)],
                                    xT[:, e, NW * n:NW * (n + 1)],
                                    start=(e == 0), stop=(e == NE - 1))
                            gsb = gp.tile([128, NW], F32, tag="gsb", bufs=3)
                            nc.scalar.activation(
                                gsb[:, :], gps[:, :], AF.Identity,
                                bias=bgi[:, j:j + 1])
                            nc.sync.dma_start(
                                giT_d.ap()[:, j, NW * n:NW * (n + 1)], gsb[:, :])
                wp_ctx.__exit__(None, None, None)

            # ---------- phase E: recurrence (bf16 weights/stream) ----------
            with tc.tile_pool(name="rec", bufs=1) as rp:
                whh = rp.tile([128, NK, G3], BF16, tag="whh")
                for k in range(NK):
                    wsc2 = rp.tile([128, G3], F32, tag="wsc2", bufs=2)
                    nc.sync.dma_start(
                        wsc2[:, :], whhT_d.ap()[128 * k:128 * (k + 1), :])
                    nc.vector.tensor_copy(whh[:, k, :], wsc2[:, :])

                HB = BL * NK // 2          # 40: half of the hc free dim
                with tc.tile_pool(name="recw", bufs=4) as rw, \
                     tc.tile_pool(name="hpool", bufs=3) as hp, \
                     tc.tile_pool(name="grz_ps", bufs=4, space="PSUM") as rzp, \
                     tc.tile_pool(name="gn_ps", bufs=4, space="PSUM") as gnp:
                    h = hp.tile([128, BL * NK], F32, tag="h")
                    nc.vector.memset(h[:, :], 0.0)
                    hc_a = hp.tile([128, HB], BF16, tag="hca")
                    nc.vector.memset(hc_a[:, :], 0.0)
                    hc_b = hp.tile([128, HB], BF16, tag="hcb")
                    nc.vector.memset(hc_b[:, :], 0.0)
                    for t in range(T):
                        gi_t = rw.tile([128, NJ, BL], F32, tag="gi")
                        nc.sync.dma_start(
                            gi_t[:, :, :], giT_d.ap()[:, :, BL * t:BL * (t + 1)])
                        G_rz = rzp.tile([128, 20 * BL], F32, tag="grz")
                        G_n = gnp.tile([128, 10 * BL], F32, tag="gn")

                        # gate-group emission order r -> n -> z, with the
                        # contraction split over the two hc halves so the
                        # first matmuls only gate on hc_a from last step.
                        def gate_mms(jlo):
                            for j in range(jlo, jlo + 10):
                                sl = (G_rz[:, BL * j:BL * (j + 1)]
                                      if j < 20
                                      else G_n[:, BL * (j - 20):
                                               BL * (j - 19)])
                                for k in range(NK):
                                    hc_h = hc_a if k < 5 else hc_b
                                    nc.tensor.matmul(
                                        sl,
                                        whh[:, k, 128 * j:128 * (j + 1)],
                                        hc_h[:, BL * (k % 5):
                                             BL * (k % 5 + 1)],
                                        start=(k == 0), stop=(k == NK - 1))

                        gate_mms(0)
                        # r-path overlaps the n/z matmuls below
                        r_pre = rw.tile([128, 10 * BL], F32, tag="rpre")
                        nc.vector.tensor_tensor(
                            r_pre[:, :], G_rz[:, 0:10 * BL],
                            gi_t[:, 0:10, :].rearrange("p j b -> p (j b)"), OP.add)
                        r_g = rw.tile([128, 10 * BL], F32, tag="rg")
                        nc.scalar.activation(r_g[:, :], r_pre[:, :], AF.Sigmoid)

                        gate_mms(20)
                        t1 = rw.tile([128, 10 * BL], F32, tag="t1")
                        nc.vector.tensor_tensor(
                            t1[:, :], G_n[:, :], bhhn[:, :], OP.add)
                        t2 = rw.tile([128, 10 * BL], F32, tag="t2")
                        nc.vector.tensor_tensor(
                            t2[:, :], t1[:, :], r_g[:, :], OP.mult)
                        npre = rw.tile([128, 10 * BL], F32, tag="npre")
                        nc.vector.tensor_tensor(
                            npre[:, :], t2[:, :],
                            gi_t[:, 20:30, :].rearrange("p j b -> p (j b)"), OP.add)
                        n_g = rw.tile([128, 10 * BL], F32, tag="ng")
                        nc.scalar.activation(n_g[:, :], npre[:, :], AF.Tanh)
                        w_t = rw.tile([128, 10 * BL], F32, tag="wt")
                        nc.vector.tensor_tensor(
                            w_t[:, :], h[:, :], n_g[:, :], OP.subtract)

                        gate_mms(10)
                        # exposed tail: z_pre -> sigmoid -> u2 -> h'
                        z_pre = rw.tile([128, 10 * BL], F32, tag="zpre")
                        nc.vector.tensor_tensor(
                            z_pre[:, :], G_rz[:, 10 * BL:20 * BL],
                            gi_t[:, 10:20, :].rearrange("p j b -> p (j b)"), OP.add)
                        z_g = rw.tile([128, 10 * BL], F32, tag="zg")
                        nc.scalar.activation(z_g[:, :], z_pre[:, :], AF.Sigmoid)
                        u2 = rw.tile([128, 10 * BL], F32, tag="u2")
                        nc.vector.tensor_tensor(
                            u2[:, :], z_g[:, :], w_t[:, :], OP.mult)
                        hc_a = hp.tile([128, HB], BF16, tag="hca")
                        nc.vector.tensor_tensor(
                            hc_a[:, :], n_g[:, 0:HB], u2[:, 0:HB], OP.add)
                        hc_b = hp.tile([128, HB], BF16, tag="hcb")
                        nc.vector.tensor_tensor(
                            hc_b[:, :], n_g[:, HB:2 * HB], u2[:, HB:2 * HB],
                            OP.add)
                        h = hp.tile([128, BL * NK], F32, tag="h")
                        nc.vector.tensor_tensor(
                            h[:, :], n_g[:, :], u2[:, :], OP.add)
                        nc.sync.dma_start(outT_d.ap()[t, :, :], h[:, :])


class _Compiled:
    def __init__(self):
        import jax
        import numpy as _np
        import concourse.bacc as bacc
        import concourse.mybir as mybir
        from jax.sharding import Mesh, PartitionSpec, NamedSharding
        from jax.experimental.shard_map import shard_map
        from concourse.bass2jax import (
            _bass_exec_p, partition_id_tensor, install_neuronx_cc_hook)

        install_neuronx_cc_hook()
        nc = bacc.Bacc("TRN2", target_bir_lowering=False, debug=False,
                       enable_asserts=True, num_devices=1)
        _build(nc)
        nc.compile()
        self.nc = nc
        self.jax = jax

        partition_name = (nc.partition_id_tensor.name
                          if nc.partition_id_tensor else None)
        in_names, out_names, out_avals, zero_outs = [], [], [], []
        for alloc in nc.m.functions[0].allocations:
            if not isinstance(alloc, mybir.MemoryLocationSet):
                continue
            name = alloc.memorylocations[0].name
            if alloc.kind == "ExternalInput":
                if name != partition_name:
                    in_names.append(name)
            elif alloc.kind == "ExternalOutput":
                out_names.append(name)
                shape = tuple(alloc.tensor_shape)
                dt = mybir.dt.np(alloc.dtype)
                out_avals.append(jax.core.ShapedArray(shape, dt))
                zero_outs.append(_np.zeros(shape, dt))
        self.in_params = list(in_names)
        self.out_names = out_names
        self.out_avals = out_avals
        n_params = len(in_names)
        in_names = in_names + out_names
        if partition_name is not None:
            in_names.append(partition_name)

        def _body(*args):
            args = list(args)
            if partition_name is not None:
                args.append(partition_id_tensor())
            outs = _bass_exec_p.bind(
                *args, out_avals=tuple(out_avals), in_names=tuple(in_names),
                out_names=tuple(out_names), lowering_input_output_aliases=(),
                sim_require_finite=True, sim_require_nnan=True, nc=nc)
            return tuple(outs)

        devices = jax.devices()[:N_CORES]
        mesh = Mesh(_np.asarray(devices), ("core",))
        n_in = n_params + len(out_names)
        self.sharded = jax.jit(
            shard_map(_body, mesh=mesh,
                      in_specs=(PartitionSpec("core"),) * n_in,
                      out_specs=(PartitionSpec("core"),) * len(out_names),
                      check_rep=False),
            keep_unused=True)
        self.sh = NamedSharding(mesh, PartitionSpec("core"))
        self.zero_outs = zero_outs

    def put_inputs(self, in_maps):
        import numpy as _np
        jax = self.jax
        concat = [_np.concatenate([_np.ascontiguousarray(in_maps[c][n])
                                   for c in range(N_CORES)], axis=0)
                  for n in self.in_params]
        args = [jax.device_put(a, self.sh) for a in concat]
        zeros = [jax.device_put(
            _np.zeros((N_CORES * z.shape[0], *z.shape[1:]), z.dtype), self.sh)
            for z in self.zero_outs]
        return args + zeros

    def run(self, dev_args):
        out = self.sharded(*dev_args)
        self.jax.block_until_ready(out)
        return out

    def results(self, out):
        import numpy as _np
        res = []
        for c in range(N_CORES):
            d = {}
            for i, name in enumerate(self.out_names):
                a = _np.asarray(out[i])
                d[name] = a.reshape(N_CORES, *self.out_avals[i].shape)[c]
            res.append(d)
        return res


def _get_compiled():
    if "k" not in _CACHE:
        _CACHE["k"] = _Compiled()
    return _CACHE["k"]


def _prep_core_inputs(source_core, embedding, wihT, whhT, bgi, bhhn, ident):
    NTOK = T * BL
    idx_lin = source_core.T.reshape(-1)          # t-major: i = t*8 + b
    idx = np.tile(idx_lin.reshape(NTOK // 16, 16).T, (8, 1)).astype(np.int16)
    return {"emb": embedding, "idx": idx, "wihT": wihT, "whhT": whhT,
            "bgi": bgi, "bhhn": bhhn, "ident": ident}


def prep_in_maps(source, embedding, w_ih, w_hh, b_ih, b_hh):
    source = np.asarray(source)
    embedding = np.ascontiguousarray(np.asarray(embedding, dtype=np.float32))
    w_ih = np.asarray(w_ih, dtype=np.float32)
    w_hh = np.asarray(w_hh, dtype=np.float32)
    b_ih = np.asarray(b_ih, dtype=np.float32)
    b_hh = np.asarray(b_hh, dtype=np.float32)
    wihT = np.ascontiguousarray(w_ih.T)
    whhT = np.ascontiguousarray(w_hh.T)
    bias_gi = np.concatenate([(b_ih + b_hh)[:2 * H], b_ih[2 * H:]])
    bgi = np.ascontiguousarray(bias_gi.reshape(NJ, 128).T, dtype=np.float32)
    bhh_n = b_hh[2 * H:]
    bhhn = np.ascontiguousarray(
        np.repeat(bhh_n.reshape(NK, 128).T[:, :, None], BL, axis=2)
        .reshape(128, NK * BL), dtype=np.float32)
    ident = np.eye(128, dtype=np.float32)
    return [
        _prep_core_inputs(source[c * BL:(c + 1) * BL], embedding, wihT, whhT,
                          bgi, bhhn, ident)
        for c in range(N_CORES)]


def unpack_results(res):
    """res: list of per-core {'outT': [T, 128, 80]} -> [T, B, H] float32."""
    outs = []
    for c in range(N_CORES):
        o = res[c]["outT"].reshape(T, 128, NK, BL)
        outs.append(o.transpose(0, 3, 2, 1).reshape(T, BL, H))
    return np.concatenate(outs, axis=1).astype(np.float32)


def kernel(source, embedding, w_ih, w_hh, b_ih, b_hh):
    k = _get_compiled()
    in_maps = prep_in_maps(source, embedding, w_ih, w_hh, b_ih, b_hh)
    dev_args = k.put_inputs(in_maps)
    out = k.run(dev_args)
    return unpack_results(k.results(out))

